# revision 47
# baseline (speedup 1.0000x reference)
"""Trainium2 Bass kernel for nn_BatchProgramClassifier.

Reference computation (B=64, L=64, NPT=127, D=128, VOCAB=30000, LABELS=30):
  1. e = emb[tokens] @ Wc + bc                     per tree node
  2. h = bottom-up subtree sums of e (heap tree)   [B, L, NPT, D]
  3. enc = relu(max over nodes of h)               [B, L, D]
  4. masked single-head self-attention over L      [B, L, D]
  5. logits = (max over L) @ Wl + bl               [B, LABELS]

Sharding: data-parallel over batch, 8 batches per core across 8 cores.

Device pipeline (per core, all phases overlap under the gather):
  - 32 dma_gather chunks (2048 rows each) pull fp16 embedding rows from HBM
    in transpose mode straight into D-major layout, 4 SWDGE queues deep.
  - Tree nodes are stored in bit-reversed level order (host permutes the
    index array): level l occupies slots [2^l-1, 2^(l+1)-1) and the children
    of slot-i parents are the two contiguous half-blocks of the next level.
    Subtree sums and the node-max then run on DVE with fully contiguous
    innermost slices (2x 16-bit mode), the max as a log-depth tensor_max
    tree instead of the slow TENSOR_REDUCE path.
  - Wc is folded in per 512-column matmul with the +bc bias applied during
    the PSUM->SBUF copy on ACT.
  - Attention runs per batch (64 trees) as soon as that batch's encoder
    columns are done, so only the last batch's attention sits in the tail.
"""

import math

import numpy as np

B, L, NPT, D_TREE = 64, 64, 127, 7
VOCAB, D, LABELS = 30000, 128, 30
NCORES = 8
BC = B // NCORES  # batches per core
TREES = BC * L  # trees per core (512)
SLOTS = 128  # per-tree storage (127 nodes + 1 pad)
CHUNK_TREES = 16  # trees per gather chunk
NCHUNKS = TREES // CHUNK_TREES  # 32
NIDX_CHUNK = CHUNK_TREES * SLOTS  # 2048
NIDX_TOTAL = TREES * SLOTS  # 65536
CHUNKS_PER_BATCH = L // CHUNK_TREES  # 4

_CACHE = {}


def _bitrev_slots():
    """Storage slot for each heap node 0..126: level l occupies slots
    [2^l, 2^(l+1)) with bit-reversed within-level order, slot 0 is the pad.
    Children of slot 2^l + i are slots 2^(l+1) + i and 2^(l+1) + 2^l + i,
    i.e. two contiguous half-blocks, and all level blocks are power-of-two
    aligned (chunk-aligned for the gather)."""
    slots = np.zeros(NPT, np.int64)
    for h in range(NPT):
        lvl = (h + 1).bit_length() - 1
        j = h - (2**lvl - 1)
        r = 0
        for b in range(lvl):
            r = (r << 1) | ((j >> b) & 1)
        slots[h] = 2**lvl + r
    return slots


def _build_nc():
    import concourse.bacc as bacc
    import concourse.mybir as mybir
    import concourse.tile as tile
    from concourse.library_config import mlp

    f32 = mybir.dt.float32
    f16 = mybir.dt.float16
    nc = bacc.Bacc(
        "TRN2",
        target_bir_lowering=False,
        debug=False,
        num_devices=NCORES,
        num_swdge_queues=4,
    )

    emb_d = nc.dram_tensor("emb", [VOCAB, D], f16, kind="ExternalInput")
    idx_d = nc.dram_tensor(
        "idxs", [128, NIDX_TOTAL // 16], mybir.dt.int16, kind="ExternalInput"
    )
    nmaskT_d = nc.dram_tensor("nmaskT", [L, BC * L], f16, kind="ExternalInput")
    wc_d = nc.dram_tensor("wc", [D, D], f32, kind="ExternalInput")
    bcv_d = nc.dram_tensor("bcv", [D, 1], f32, kind="ExternalInput")
    wq_d = nc.dram_tensor("wq", [D, D], f32, kind="ExternalInput")
    wk_d = nc.dram_tensor("wk", [D, D], f32, kind="ExternalInput")
    wv_d = nc.dram_tensor("wv", [D, D], f32, kind="ExternalInput")
    wo_d = nc.dram_tensor("wo", [D, D], f32, kind="ExternalInput")
    wl_d = nc.dram_tensor("wl", [D, LABELS], f32, kind="ExternalInput")
    blb_d = nc.dram_tensor("blb", [BC, LABELS], f32, kind="ExternalInput")
    ident_d = nc.dram_tensor("ident", [128, 128], f32, kind="ExternalInput")
    out_d = nc.dram_tensor("out", [BC, LABELS], f32, kind="ExternalOutput")

    inv_sqrt_d = 1.0 / math.sqrt(float(D))

    with tile.TileContext(nc) as tc:
        with (
            tc.tile_pool(name="const", bufs=1) as cpool,
            tc.tile_pool(name="epool", bufs=10) as epool,
            tc.tile_pool(name="eblk", bufs=5) as eblkpool,
            tc.tile_pool(name="tpsum", bufs=4, space="PSUM") as tpsum,
            tc.tile_pool(name="apool", bufs=3) as apool,
            tc.tile_pool(name="apsum", bufs=2, space="PSUM") as apsum,
        ):
            nc.gpsimd.load_library(mlp)

            idx_t = cpool.tile([128, NIDX_TOTAL // 16], mybir.dt.int16, tag="idxs")
            idx_batch_cols = NIDX_TOTAL // 16 // BC
            for b in range(BC):
                nc.sync.dma_start(
                    out=idx_t[:, b * idx_batch_cols : (b + 1) * idx_batch_cols],
                    in_=idx_d[:, b * idx_batch_cols : (b + 1) * idx_batch_cols],
                )

            def load_const(dram, shape, dtype):
                t = cpool.tile(shape, dtype, tag=dram.name)
                if dtype == dram.dtype:
                    nc.sync.dma_start(out=t[:], in_=dram[:])
                else:
                    raw = cpool.tile(shape, dram.dtype, tag=dram.name + "_raw")
                    nc.sync.dma_start(out=raw[:], in_=dram[:])
                    nc.scalar.copy(out=t[:], in_=raw[:])
                return t

            wc_t = load_const(wc_d, [D, D], f16)
            bcv_t = load_const(bcv_d, [D, 1], f32)
            wq_t = load_const(wq_d, [D, D], f16)
            wk_t = load_const(wk_d, [D, D], f16)
            wv_t = load_const(wv_d, [D, D], f16)
            wo_t = load_const(wo_d, [D, D], f16)
            wl_t = load_const(wl_d, [D, LABELS], f16)
            blb_t = load_const(blb_d, [BC, LABELS], f32)
            ident_t = load_const(ident_d, [128, 128], f32)
            nmaskT_t = load_const(nmaskT_d, [L, BC * L], f16)
            ident16 = cpool.tile([128, 128], f16, tag="ident16")
            nc.scalar.copy(out=ident16[:], in_=ident_t[:])

            pooled_all = cpool.tile([D, BC], f16, tag="pooled")
            # per-batch attention outputs, max-pooled once at the end
            oall = cpool.tile([D, BC * L], f16, tag="oall")

            idx_cols = NIDX_CHUNK // 16

            def attention(b, enc_b):
                """enc_b: [D, L] f16 columns for batch b.

                PSUM tiles are column-slices of two shared banks to stay
                within the 8-bank budget alongside the Wc pipeline.
                """
                bankA = apsum.tile([D, 512], f32, tag="bankA")
                bankB = apsum.tile([D, 512], f32, tag="bankB")
                qp = bankA[:, 0:L]
                kp = bankA[:, L : 2 * L]
                vp = bankA[:, 2 * L : 3 * L]
                scp = bankA[:L, 3 * L : 4 * L]
                atp = bankA[:L, 4 * L : 5 * L]
                vtp = bankB[:L, 0:D]
                op = bankB[:, D : D + L]
                o2p = bankB[:, D + L : D + 2 * L]

                nc.tensor.matmul(qp, lhsT=wq_t[:], rhs=enc_b, start=True, stop=True)
                nc.tensor.matmul(kp, lhsT=wk_t[:], rhs=enc_b, start=True, stop=True)
                # v^T directly: vtp[l, d'] = sum_d enc[d, l] Wv[d, d']
                nc.tensor.matmul(vtp, lhsT=enc_b, rhs=wv_t[:], start=True, stop=True)
                qs = apool.tile([D, L], f16, tag="qs")
                nc.scalar.mul(qs[:], qp, inv_sqrt_d)
                ks = apool.tile([D, L], f16, tag="ks")
                nc.scalar.copy(out=ks[:], in_=kp)

                # scores with the additive mask folded in via a constant
                # matmul (nmaskT holds 0 / -3e4 transposed), so exp() off the
                # PSUM needs no mask op and its accumulator yields the row
                # sums directly -- the only DVE op left here is reciprocal.
                nc.tensor.matmul(scp, lhsT=qs[:], rhs=ks[:], start=True, stop=False)
                nc.tensor.matmul(
                    scp,
                    lhsT=nmaskT_t[:, b * L : (b + 1) * L],
                    rhs=ident16[:L, :L],
                    start=False,
                    stop=True,
                )
                ex = apool.tile([L, L], f32, tag="ex")
                rsum = apool.tile([L, 1], f32, tag="rsum")
                nc.scalar.activation(
                    ex[:],
                    scp,
                    mybir.ActivationFunctionType.Exp,
                    accum_out=rsum[:],
                )
                rinv = apool.tile([L, 1], f32, tag="rinv")
                nc.vector.reciprocal(rinv[:], rsum[:])
                attn = apool.tile([L, L], f32, tag="attn")
                nc.scalar.activation(
                    attn[:],
                    ex[:],
                    mybir.ActivationFunctionType.Identity,
                    scale=rinv[:],
                )

                nc.tensor.transpose(atp, attn[:], ident_t[:L, :L])
                ats = apool.tile([L, L], f16, tag="ats")
                nc.scalar.copy(out=ats[:], in_=atp)
                vts = apool.tile([L, D], f16, tag="vts")
                nc.scalar.copy(out=vts[:], in_=vtp)
                nc.tensor.matmul(op, lhsT=vts[:], rhs=ats[:], start=True, stop=True)
                os_ = apool.tile([D, L], f16, tag="os")
                nc.scalar.copy(out=os_[:], in_=op)
                nc.tensor.matmul(o2p, lhsT=wo_t[:], rhs=os_[:], start=True, stop=True)
                nc.scalar.copy(out=oall[:, b * L : (b + 1) * L], in_=o2p)

            def psum_to_eb(eb, pp, a, bb):
                """PSUM -> eb[a:bb) f16 with the +bc bias folded on ACT."""
                nc.scalar.activation(
                    eb[:, a:bb],
                    pp[:, 0 : bb - a],
                    mybir.ActivationFunctionType.Identity,
                    bias=bcv_t[:],
                    scale=1.0,
                )

            for b in range(BC):
                # one eb super-tile per batch: 64 trees x 128 slots, f16
                eb = eblkpool.tile([128, L * SLOTS], f16, tag="eb")
                # chunk 0 last: the first tree add (level 5) only needs
                # slots 32..127, so it can start while chunk 0 gathers
                for k in (1, 2, 3, 0):
                    c = b * CHUNKS_PER_BATCH + k
                    et = epool.tile([128, 1, NIDX_CHUNK], f16, tag="et")
                    nc.gpsimd.dma_gather(
                        et[:],
                        emb_d[:],
                        idx_t[:, c * idx_cols : (c + 1) * idx_cols],
                        NIDX_CHUNK,
                        NIDX_CHUNK,
                        D,
                        transpose=True,
                        single_packet=False,
                        queue_num=c % 4,
                    )
                    for j in range(NIDX_CHUNK // 512):
                        pp = tpsum.tile([128, 512], f32, tag="pp")
                        nc.tensor.matmul(
                            pp[:],
                            lhsT=wc_t[:],
                            rhs=et[:, 0, j * 512 : (j + 1) * 512],
                            start=True,
                            stop=True,
                        )
                        off = k * NIDX_CHUNK + j * 512
                        psum_to_eb(eb, pp, off, off + 512)

                # bottom-up subtree sums in slot-major layout: column s*L + t
                # holds slot s of tree t, every slice fully contiguous. The
                # level-5 adds and the leaf max-fold only need chunks 1-3, so
                # they run while chunk 0 (slots 0..31) is still gathering.
                for lvl in range(D_TREE - 2, -1, -1):
                    p0 = (2**lvl) * L
                    c0 = (2 ** (lvl + 1)) * L
                    w = (2**lvl) * L
                    nc.vector.tensor_add(
                        out=eb[:, p0 : p0 + w],
                        in0=eb[:, p0 : p0 + w],
                        in1=eb[:, c0 : c0 + w],
                    )
                    nc.vector.tensor_add(
                        out=eb[:, p0 : p0 + w],
                        in0=eb[:, p0 : p0 + w],
                        in1=eb[:, c0 + w : c0 + 2 * w],
                    )
                    if lvl == D_TREE - 2:
                        # leaves are dead once level 5 consumed them: fold
                        # their max in place down to slot 64 (pre-chunk-0)
                        s = 32
                        while s >= 1:
                            nc.vector.tensor_max(
                                out=eb[:, 64 * L : (64 + s) * L],
                                in0=eb[:, 64 * L : (64 + s) * L],
                                in1=eb[:, (64 + s) * L : (64 + 2 * s) * L],
                            )
                            s //= 2
                        # neutralize the pad slot (0) for the internal fold
                        nc.vector.memset(eb[:, 0:L], -1e9)
                # fold the internal slots 0..63 down to slot 0, then combine
                # with the leaf max sitting at slot 64 and apply ReLU
                s = 32
                while s >= 1:
                    nc.vector.tensor_max(
                        out=eb[:, 0 : s * L],
                        in0=eb[:, 0 : s * L],
                        in1=eb[:, s * L : 2 * s * L],
                    )
                    s //= 2
                nc.vector.tensor_max(
                    out=eb[:, 0:L], in0=eb[:, 0:L], in1=eb[:, 64 * L : 65 * L]
                )
                enc_b = apool.tile([D, L], f16, tag="encb")
                nc.vector.tensor_scalar_max(enc_b[:], eb[:, 0:L], 0.0)

                attention(b, enc_b[:])

            # ---- pooled max + logits ----
            nc.vector.reduce_max(
                out=pooled_all[:],
                in_=oall.rearrange("d (b l) -> d b l", l=L),
                axis=mybir.AxisListType.X,
            )
            lbank = apsum.tile([D, 512], f32, tag="bankA")
            lgp = lbank[:BC, :LABELS]
            nc.tensor.matmul(
                lgp[:], lhsT=pooled_all[:], rhs=wl_t[:], start=True, stop=True
            )
            outs = apool.tile([BC, LABELS], f32, tag="outs")
            nc.vector.tensor_add(out=outs[:], in0=lgp[:], in1=blb_t[:])
            nc.sync.dma_start(out=out_d[:], in_=outs[:])

    nc.compile()
    return nc


def _get_nc():
    if "nc" not in _CACHE:
        _CACHE["nc"] = _build_nc()
    return _CACHE["nc"]


def kernel(tokens, mask, emb, Wc, bc, Wq, Wk, Wv, Wo, Wl, bl, _trace=False):
    from concourse.bass_utils import run_bass_kernel_spmd

    tokens = np.asarray(tokens)
    mask = np.asarray(mask)
    emb16 = np.asarray(emb, dtype=np.float32).astype(np.float16)

    blb = np.tile(np.asarray(bl, np.float32)[None, :], (BC, 1))

    common = {
        "emb": emb16,
        "wc": np.asarray(Wc, np.float32),
        "bcv": np.asarray(bc, np.float32).reshape(D, 1),
        "wq": np.asarray(Wq, np.float32),
        "wk": np.asarray(Wk, np.float32),
        "wv": np.asarray(Wv, np.float32),
        "wo": np.asarray(Wo, np.float32),
        "wl": np.asarray(Wl, np.float32),
        "blb": blb,
        "ident": np.eye(128, dtype=np.float32),
    }

    slots = _bitrev_slots()  # heap node -> storage slot

    in_maps = []
    for c in range(NCORES):
        tok_c = np.asarray(tokens[c * BC : (c + 1) * BC]).reshape(TREES, NPT)
        slotted = np.zeros((TREES, SLOTS), tok_c.dtype)
        slotted[:, slots] = tok_c  # pad slot 127 keeps idx 0
        # slot-major within each batch: position (b, s, t) = b*8192 + s*64 + t
        idx_lin = (
            slotted.reshape(BC, L, SLOTS).transpose(0, 2, 1).reshape(-1)
        )
        idx_arr = np.tile(
            idx_lin.astype(np.int16).reshape(-1, 16).T, (8, 1)
        )  # [128, NIDX_TOTAL/16]
        # nmaskT[k, b*L + q] = 0 where mask[b, q, k] > 0 else -3e4
        mask_c = np.asarray(mask[c * BC : (c + 1) * BC]) > 0
        nmaskT = ((mask_c.astype(np.float32) - 1.0) * 3e4).transpose(2, 0, 1)
        nmaskT = np.ascontiguousarray(nmaskT.reshape(L, BC * L)).astype(np.float16)
        in_maps.append({**common, "idxs": idx_arr, "nmaskT": nmaskT})

    nc = _get_nc()
    res = run_bass_kernel_spmd(
        nc, in_maps, core_ids=list(range(NCORES)), trace=_trace
    )
    out = np.concatenate([r["out"] for r in res.results], axis=0)  # [B, LABELS]
    if _trace:
        return out, res
    return out


# revision 49
# speedup vs baseline: 1.1499x; 1.1499x over previous
"""Trainium2 Bass kernel for nn_BatchProgramClassifier.

Reference computation (B=64, L=64, NPT=127, D=128, VOCAB=30000, LABELS=30):
  1. e = emb[tokens] @ Wc + bc                     per tree node
  2. h = bottom-up subtree sums of e (heap tree)   [B, L, NPT, D]
  3. enc = relu(max over nodes of h)               [B, L, D]
  4. masked single-head self-attention over L      [B, L, D]
  5. logits = (max over L) @ Wl + bl               [B, LABELS]

Sharding: data-parallel over batch, 8 batches per core across 8 cores.

Device pipeline (per core, all phases overlap under the gather):
  - 32 dma_gather chunks (2048 rows each) pull fp16 embedding rows from HBM
    in transpose mode straight into D-major layout, 4 SWDGE queues deep.
  - Tree nodes are stored in bit-reversed level order (host permutes the
    index array): level l occupies slots [2^l-1, 2^(l+1)-1) and the children
    of slot-i parents are the two contiguous half-blocks of the next level.
    Subtree sums and the node-max then run on DVE with fully contiguous
    innermost slices (2x 16-bit mode), the max as a log-depth tensor_max
    tree instead of the slow TENSOR_REDUCE path.
  - Wc is folded in per 512-column matmul with the +bc bias applied during
    the PSUM->SBUF copy on ACT.
  - Attention runs per batch (64 trees) as soon as that batch's encoder
    columns are done, so only the last batch's attention sits in the tail.
"""

import math

import numpy as np

B, L, NPT, D_TREE = 64, 64, 127, 7
VOCAB, D, LABELS = 30000, 128, 30
NCORES = 8
BC = B // NCORES  # batches per core
TREES = BC * L  # trees per core (512)
SLOTS = 128  # per-tree storage (127 nodes + 1 pad)
CHUNK_TREES = 16  # trees per gather chunk
NCHUNKS = TREES // CHUNK_TREES  # 32
NIDX_CHUNK = CHUNK_TREES * SLOTS  # 2048
NIDX_TOTAL = TREES * SLOTS  # 65536
CHUNKS_PER_BATCH = L // CHUNK_TREES  # 4

_CACHE = {}


def _bitrev_slots():
    """Storage slot for each heap node 0..126: level l occupies slots
    [2^l, 2^(l+1)) with bit-reversed within-level order, slot 0 is the pad.
    Children of slot 2^l + i are slots 2^(l+1) + i and 2^(l+1) + 2^l + i,
    i.e. two contiguous half-blocks, and all level blocks are power-of-two
    aligned (chunk-aligned for the gather)."""
    slots = np.zeros(NPT, np.int64)
    for h in range(NPT):
        lvl = (h + 1).bit_length() - 1
        j = h - (2**lvl - 1)
        r = 0
        for b in range(lvl):
            r = (r << 1) | ((j >> b) & 1)
        slots[h] = 2**lvl + r
    return slots


def _build_nc():
    import concourse.bacc as bacc
    import concourse.mybir as mybir
    import concourse.tile as tile
    from concourse.library_config import mlp

    f32 = mybir.dt.float32
    f16 = mybir.dt.float16
    nc = bacc.Bacc(
        "TRN2",
        target_bir_lowering=False,
        debug=False,
        num_devices=NCORES,
        num_swdge_queues=4,
    )

    emb_d = nc.dram_tensor("emb", [VOCAB, D], f16, kind="ExternalInput")
    idx_d = nc.dram_tensor(
        "idxs", [128, NIDX_TOTAL // 16], mybir.dt.int16, kind="ExternalInput"
    )
    nmaskT_d = nc.dram_tensor("nmaskT", [L, BC * L], f16, kind="ExternalInput")
    wc_d = nc.dram_tensor("wc", [D, D], f32, kind="ExternalInput")
    bcv_d = nc.dram_tensor("bcv", [D, 1], f32, kind="ExternalInput")
    wq_d = nc.dram_tensor("wq", [D, D], f32, kind="ExternalInput")
    wk_d = nc.dram_tensor("wk", [D, D], f32, kind="ExternalInput")
    wv_d = nc.dram_tensor("wv", [D, D], f32, kind="ExternalInput")
    wo_d = nc.dram_tensor("wo", [D, D], f32, kind="ExternalInput")
    wl_d = nc.dram_tensor("wl", [D, LABELS], f32, kind="ExternalInput")
    blb_d = nc.dram_tensor("blb", [BC, LABELS], f32, kind="ExternalInput")
    ident_d = nc.dram_tensor("ident", [128, 128], f32, kind="ExternalInput")
    out_d = nc.dram_tensor("out", [BC, LABELS], f32, kind="ExternalOutput")

    inv_sqrt_d = 1.0 / math.sqrt(float(D))

    with tile.TileContext(nc) as tc:
        with (
            tc.tile_pool(name="const", bufs=1) as cpool,
            tc.tile_pool(name="epool", bufs=10) as epool,
            tc.tile_pool(name="eblk", bufs=5) as eblkpool,
            tc.tile_pool(name="tpsum", bufs=4, space="PSUM") as tpsum,
            tc.tile_pool(name="apool", bufs=3) as apool,
            tc.tile_pool(name="apsum", bufs=2, space="PSUM") as apsum,
        ):
            nc.gpsimd.load_library(mlp)

            idx_t = cpool.tile([128, NIDX_TOTAL // 16], mybir.dt.int16, tag="idxs")
            idx_batch_cols = NIDX_TOTAL // 16 // BC
            for b in range(BC):
                nc.sync.dma_start(
                    out=idx_t[:, b * idx_batch_cols : (b + 1) * idx_batch_cols],
                    in_=idx_d[:, b * idx_batch_cols : (b + 1) * idx_batch_cols],
                )

            def load_const(dram, shape, dtype):
                t = cpool.tile(shape, dtype, tag=dram.name)
                if dtype == dram.dtype:
                    nc.sync.dma_start(out=t[:], in_=dram[:])
                else:
                    raw = cpool.tile(shape, dram.dtype, tag=dram.name + "_raw")
                    nc.sync.dma_start(out=raw[:], in_=dram[:])
                    nc.scalar.copy(out=t[:], in_=raw[:])
                return t

            wc_t = load_const(wc_d, [D, D], f16)
            bcv_t = load_const(bcv_d, [D, 1], f32)
            wq_t = load_const(wq_d, [D, D], f16)
            wk_t = load_const(wk_d, [D, D], f16)
            wv_t = load_const(wv_d, [D, D], f16)
            wo_t = load_const(wo_d, [D, D], f16)
            wl_t = load_const(wl_d, [D, LABELS], f16)
            blb_t = load_const(blb_d, [BC, LABELS], f32)
            ident_t = load_const(ident_d, [128, 128], f32)
            nmaskT_t = load_const(nmaskT_d, [L, BC * L], f16)
            ident16 = cpool.tile([128, 128], f16, tag="ident16")
            nc.scalar.copy(out=ident16[:], in_=ident_t[:])

            pooled_all = cpool.tile([D, BC], f16, tag="pooled")
            # encoder outputs for all batches; attention runs once, batched,
            # after the gather/tree pipeline so its cross-engine chains never
            # head-of-line-block the per-batch work in the in-order queues
            enc_all = cpool.tile([D, BC * L], f16, tag="encall")

            idx_cols = NIDX_CHUNK // 16
            W = BC * L  # 512

            def tail_attention():
                qp = apsum.tile([D, W], f32, tag="bankA")
                nc.tensor.matmul(
                    qp[:], lhsT=wq_t[:], rhs=enc_all[:], start=True, stop=True
                )
                qs = apool.tile([D, W], f16, tag="qs")
                nc.scalar.mul(qs[:], qp[:], inv_sqrt_d)
                kp = apsum.tile([D, W], f32, tag="bankB")
                nc.tensor.matmul(
                    kp[:], lhsT=wk_t[:], rhs=enc_all[:], start=True, stop=True
                )
                ks = apool.tile([D, W], f16, tag="ks")
                nc.scalar.copy(out=ks[:], in_=kp[:])

                # scores for all batches in one bank, additive mask folded in
                # via constant matmuls (nmaskT holds 0 / -3e4 transposed)
                scp = apsum.tile([D, W], f32, tag="bankA")
                for b in range(BC):
                    s = scp[:L, b * L : (b + 1) * L]
                    nc.tensor.matmul(
                        s,
                        lhsT=qs[:, b * L : (b + 1) * L],
                        rhs=ks[:, b * L : (b + 1) * L],
                        start=True,
                        stop=False,
                    )
                    nc.tensor.matmul(
                        s,
                        lhsT=nmaskT_t[:, b * L : (b + 1) * L],
                        rhs=ident16[:L, :L],
                        start=False,
                        stop=True,
                    )
                ex = apool.tile([L, W], f32, tag="ex")
                nc.scalar.activation(
                    ex[:], scp[:L, :], mybir.ActivationFunctionType.Exp
                )
                rsum = apool.tile([L, BC], f32, tag="rsum")
                nc.vector.reduce_sum(
                    out=rsum[:],
                    in_=ex.rearrange("q (b k) -> q b k", k=L),
                    axis=mybir.AxisListType.X,
                )
                rinv = apool.tile([L, BC], f32, tag="rinv")
                nc.vector.reciprocal(rinv[:], rsum[:])
                attn = apool.tile([L, W], f32, tag="attn")
                nc.vector.tensor_mul(
                    out=attn.rearrange("q (b k) -> q b k", k=L),
                    in0=ex.rearrange("q (b k) -> q b k", k=L),
                    in1=rinv[:, :, None].to_broadcast((L, BC, L)),
                )

                # v^T directly per batch: vtp[l, d'] = sum_d enc[d, l] Wv[d, d']
                vts = apool.tile([L, BC * D], f16, tag="vts")
                for half in range(2):
                    vtp = apsum.tile([D, W], f32, tag="bankB")
                    for i in range(BC // 2):
                        b = half * (BC // 2) + i
                        nc.tensor.matmul(
                            vtp[:L, i * D : (i + 1) * D],
                            lhsT=enc_all[:, b * L : (b + 1) * L],
                            rhs=wv_t[:],
                            start=True,
                            stop=True,
                        )
                    nc.scalar.copy(
                        out=vts[:, half * 4 * D : (half + 1) * 4 * D],
                        in_=vtp[:L, :],
                    )
                atp = apsum.tile([D, W], f32, tag="bankA")
                for b in range(BC):
                    nc.tensor.transpose(
                        atp[:L, b * L : (b + 1) * L],
                        attn[:, b * L : (b + 1) * L],
                        ident_t[:L, :L],
                    )
                ats = apool.tile([L, W], f16, tag="ats")
                nc.scalar.copy(out=ats[:], in_=atp[:L, :])

                op = apsum.tile([D, W], f32, tag="bankB")
                for b in range(BC):
                    nc.tensor.matmul(
                        op[:, b * L : (b + 1) * L],
                        lhsT=vts[:, b * D : (b + 1) * D],
                        rhs=ats[:, b * L : (b + 1) * L],
                        start=True,
                        stop=True,
                    )
                os_ = apool.tile([D, W], f16, tag="os")
                nc.scalar.copy(out=os_[:], in_=op[:])
                o2p = apsum.tile([D, W], f32, tag="bankA")
                nc.tensor.matmul(
                    o2p[:], lhsT=wo_t[:], rhs=os_[:], start=True, stop=True
                )
                nc.vector.reduce_max(
                    out=pooled_all[:],
                    in_=o2p.rearrange("d (b l) -> d b l", l=L),
                    axis=mybir.AxisListType.X,
                )

            def psum_to_eb(eb, pp, a, bb):
                """PSUM -> eb[a:bb) f16 with the +bc bias folded on ACT."""
                nc.scalar.activation(
                    eb[:, a:bb],
                    pp[:, 0 : bb - a],
                    mybir.ActivationFunctionType.Identity,
                    bias=bcv_t[:],
                    scale=1.0,
                )

            for b in range(BC):
                # one eb super-tile per batch: 64 trees x 128 slots, f16
                eb = eblkpool.tile([128, L * SLOTS], f16, tag="eb")
                # chunk 0 last: the first tree add (level 5) only needs
                # slots 32..127, so it can start while chunk 0 gathers
                for k in (1, 2, 3, 0):
                    c = b * CHUNKS_PER_BATCH + k
                    et = epool.tile([128, 1, NIDX_CHUNK], f16, tag="et")
                    nc.gpsimd.dma_gather(
                        et[:],
                        emb_d[:],
                        idx_t[:, c * idx_cols : (c + 1) * idx_cols],
                        NIDX_CHUNK,
                        NIDX_CHUNK,
                        D,
                        transpose=True,
                        single_packet=False,
                        queue_num=c % 4,
                    )
                    for j in range(NIDX_CHUNK // 512):
                        pp = tpsum.tile([128, 512], f32, tag="pp")
                        nc.tensor.matmul(
                            pp[:],
                            lhsT=wc_t[:],
                            rhs=et[:, 0, j * 512 : (j + 1) * 512],
                            start=True,
                            stop=True,
                        )
                        off = k * NIDX_CHUNK + j * 512
                        psum_to_eb(eb, pp, off, off + 512)

                # bottom-up subtree sums in slot-major layout: column s*L + t
                # holds slot s of tree t, every slice fully contiguous. The
                # level-5 adds and the leaf max-fold only need chunks 1-3, so
                # they run while chunk 0 (slots 0..31) is still gathering.
                for lvl in range(D_TREE - 2, -1, -1):
                    p0 = (2**lvl) * L
                    c0 = (2 ** (lvl + 1)) * L
                    w = (2**lvl) * L
                    nc.vector.tensor_add(
                        out=eb[:, p0 : p0 + w],
                        in0=eb[:, p0 : p0 + w],
                        in1=eb[:, c0 : c0 + w],
                    )
                    nc.vector.tensor_add(
                        out=eb[:, p0 : p0 + w],
                        in0=eb[:, p0 : p0 + w],
                        in1=eb[:, c0 + w : c0 + 2 * w],
                    )
                    if lvl == D_TREE - 2:
                        # leaves are dead once level 5 consumed them: fold
                        # their max in place down to slot 64 (pre-chunk-0)
                        s = 32
                        while s >= 1:
                            nc.vector.tensor_max(
                                out=eb[:, 64 * L : (64 + s) * L],
                                in0=eb[:, 64 * L : (64 + s) * L],
                                in1=eb[:, (64 + s) * L : (64 + 2 * s) * L],
                            )
                            s //= 2
                        # neutralize the pad slot (0) for the internal fold
                        nc.vector.memset(eb[:, 0:L], -1e9)
                # fold the internal slots 0..63 down to slot 0, then combine
                # with the leaf max sitting at slot 64 and apply ReLU
                s = 32
                while s >= 1:
                    nc.vector.tensor_max(
                        out=eb[:, 0 : s * L],
                        in0=eb[:, 0 : s * L],
                        in1=eb[:, s * L : 2 * s * L],
                    )
                    s //= 2
                nc.vector.tensor_max(
                    out=eb[:, 0:L], in0=eb[:, 0:L], in1=eb[:, 64 * L : 65 * L]
                )
                nc.vector.tensor_scalar_max(
                    enc_all[:, b * L : (b + 1) * L], eb[:, 0:L], 0.0
                )

            # ---- batched attention + logits ----
            tail_attention()
            lbank = apsum.tile([D, 512], f32, tag="bankA")
            lgp = lbank[:BC, :LABELS]
            nc.tensor.matmul(
                lgp[:], lhsT=pooled_all[:], rhs=wl_t[:], start=True, stop=True
            )
            outs = apool.tile([BC, LABELS], f32, tag="outs")
            nc.vector.tensor_add(out=outs[:], in0=lgp[:], in1=blb_t[:])
            nc.sync.dma_start(out=out_d[:], in_=outs[:])

    nc.compile()
    return nc


def _get_nc():
    if "nc" not in _CACHE:
        _CACHE["nc"] = _build_nc()
    return _CACHE["nc"]


def kernel(tokens, mask, emb, Wc, bc, Wq, Wk, Wv, Wo, Wl, bl, _trace=False):
    from concourse.bass_utils import run_bass_kernel_spmd

    tokens = np.asarray(tokens)
    mask = np.asarray(mask)
    emb16 = np.asarray(emb, dtype=np.float32).astype(np.float16)

    blb = np.tile(np.asarray(bl, np.float32)[None, :], (BC, 1))

    common = {
        "emb": emb16,
        "wc": np.asarray(Wc, np.float32),
        "bcv": np.asarray(bc, np.float32).reshape(D, 1),
        "wq": np.asarray(Wq, np.float32),
        "wk": np.asarray(Wk, np.float32),
        "wv": np.asarray(Wv, np.float32),
        "wo": np.asarray(Wo, np.float32),
        "wl": np.asarray(Wl, np.float32),
        "blb": blb,
        "ident": np.eye(128, dtype=np.float32),
    }

    slots = _bitrev_slots()  # heap node -> storage slot

    in_maps = []
    for c in range(NCORES):
        tok_c = np.asarray(tokens[c * BC : (c + 1) * BC]).reshape(TREES, NPT)
        slotted = np.zeros((TREES, SLOTS), tok_c.dtype)
        slotted[:, slots] = tok_c  # pad slot 127 keeps idx 0
        # slot-major within each batch: position (b, s, t) = b*8192 + s*64 + t
        idx_lin = (
            slotted.reshape(BC, L, SLOTS).transpose(0, 2, 1).reshape(-1)
        )
        idx_arr = np.tile(
            idx_lin.astype(np.int16).reshape(-1, 16).T, (8, 1)
        )  # [128, NIDX_TOTAL/16]
        # nmaskT[k, b*L + q] = 0 where mask[b, q, k] > 0 else -3e4
        mask_c = np.asarray(mask[c * BC : (c + 1) * BC]) > 0
        nmaskT = ((mask_c.astype(np.float32) - 1.0) * 3e4).transpose(2, 0, 1)
        nmaskT = np.ascontiguousarray(nmaskT.reshape(L, BC * L)).astype(np.float16)
        in_maps.append({**common, "idxs": idx_arr, "nmaskT": nmaskT})

    nc = _get_nc()
    res = run_bass_kernel_spmd(
        nc, in_maps, core_ids=list(range(NCORES)), trace=_trace
    )
    out = np.concatenate([r["out"] for r in res.results], axis=0)  # [B, LABELS]
    if _trace:
        return out, res
    return out
